# revision 30
# baseline (speedup 1.0000x reference)
"""GCN layer kernel for 8 trn2 NeuronCores.

Math:  out = D (A + I) D feature W^T + b      (D = diag(hat_d))
Rewritten with g = (hat_d * feature) @ W^T  (the linear commutes with the
row-scaling and the SpMM):
    out = hat_d * (A @ g) + hat_d * g + b

Sharding: A row-sharded across 8 cores (2048 rows each). Each core
computes full g locally from a replicated feature^T (N*d is small vs N^2,
so replicating this small compute is cheaper than a collective), then
streams its A-shard once for the big matmul.

Device layout: the big matmul is computed transposed,
out_sh^T[o, m] = sum_j g[j, o] * A_sh^T[j, m], so g tiles are the
stationary operand and the A shard (pre-transposed on the host — lhsT
layout prep for the systolic array) is the moving operand in natural
layout. The host applies an "own rows first" node permutation to the j
axis of A^T / feature^T / hat_d so the same SPMD program works on every
core (own-shard g tiles are always j = 0..15).

Matmul operands are fp16 (10-bit mantissa, ~= the PE's fp32r 11-bit
precision) which streams at full PE rate and halves A's HBM traffic;
all accumulation and the epilogue stay fp32. Measured end-to-end
relative error ~4e-4.
"""

import os

import numpy as np

import concourse.mybir as mybir
import concourse.tile as tile
from concourse import bacc
from concourse.bass_utils import run_bass_kernel_spmd
from concourse.masks import make_identity

N = 16384
F = 512  # in features
O = 256  # out features
NCORES = 8
SH = N // NCORES  # 2048 rows per core
JT = N // 128  # 128 node tiles
MT = SH // 128  # 16 own node tiles
NB = 2048  # phase-1 node-block width (per feature slab)

F32 = mybir.dt.float32
F16 = mybir.dt.float16

_CACHE = {}


def build_program():
    nc = bacc.Bacc("TRN2", target_bir_lowering=False, debug=False,
                   num_devices=NCORES, dynamic_dma_scratch_size=8192)

    at = nc.dram_tensor("at", [N, SH], F16, kind="ExternalInput").ap()
    ft = nc.dram_tensor("ft", [F, N], F16, kind="ExternalInput").ap()
    hdt = nc.dram_tensor("hdt", [128, JT], F32, kind="ExternalInput").ap()
    hdo = nc.dram_tensor("hdo", [1, SH], F32, kind="ExternalInput").ap()
    wt = nc.dram_tensor("wt", [F, O], F16, kind="ExternalInput").ap()
    bvec = nc.dram_tensor("bvec", [O, 1], F32, kind="ExternalInput").ap()
    outT = nc.dram_tensor("outT", [O, SH], F32, kind="ExternalOutput").ap()

    add = mybir.AluOpType.add
    mult = mybir.AluOpType.mult

    with tile.TileContext(nc) as tc:
        with (
            tc.tile_pool(name="const", bufs=1) as constp,
            tc.tile_pool(name="gpool", bufs=1) as gp,
            tc.tile_pool(name="fslab", bufs=12) as fsp,
            tc.tile_pool(name="aslab", bufs=8) as asp,
            tc.tile_pool(name="tout", bufs=4) as wp,
            tc.tile_pool(name="scr", bufs=2) as scp,
        ):
            qs = [nc.sync, nc.scalar]

            # First feature block's slabs go out before the consts so
            # phase-1 compute starts as early as possible.
            first_slabs = []
            for fc in range(4):
                s = fsp.tile([128, NB], F16, tag="fs", name=f"fs0_{fc}")
                nc.sync.dma_start(out=s[:], in_=ft[fc * 128:(fc + 1) * 128, 0:NB])
                first_slabs.append(s)

            ident = constp.tile([128, 128], F32, tag="ident")
            make_identity(nc, ident[:])

            wt_sb = constp.tile([128, 4 * O], F16, tag="wt")
            for fc in range(4):
                nc.scalar.dma_start(out=wt_sb[:, fc * O:(fc + 1) * O],
                                    in_=wt[fc * 128:(fc + 1) * 128, :])
            hdt_sb = constp.tile([128, JT], F32, tag="hdt")
            nc.scalar.dma_start(out=hdt_sb[:], in_=hdt[:, :])

            # g for all nodes (fp16), node-tile j at columns [j*O, (j+1)*O)
            g_sb = gp.tile([128, JT * O], F16, tag="g")
            # e = (hat_d_own * g_own)^T (fp32), o-half h at cols [h*SH, (h+1)*SH)
            e_sb = gp.tile([128, 2 * SH], F32, tag="e")

            # ---- phase 1: g = (hat_d * feature) @ W^T for all nodes ----
            # Own PSUM pool (closed before the accumulators are allocated)
            # so the fw tiles pipeline 6 deep instead of sharing the
            # accumulator-sized slots. The hat_d row-scale alternates
            # between DVE and ACT so neither engine gates the PE stream.
            with tc.tile_pool(name="ps1", bufs=2, space="PSUM") as ps1:
                for jb in range(N // NB):
                    if jb == 0:
                        slabs = first_slabs
                    else:
                        slabs = []
                        for fc in range(4):
                            s = fsp.tile([128, NB], F16, tag="fs",
                                         name=f"fs{jb}_{fc}")
                            qs[fc % 2].dma_start(
                                out=s[:],
                                in_=ft[fc * 128:(fc + 1) * 128,
                                       jb * NB:(jb + 1) * NB])
                            slabs.append(s)
                    for jj in range(NB // 128):
                        j = jb * (NB // 128) + jj
                        pfw = ps1.tile([128, O], F32, tag="fw", bufs=6)
                        for fc in range(4):
                            nc.tensor.matmul(
                                pfw[:],
                                lhsT=slabs[fc][:, jj * 128:(jj + 1) * 128],
                                rhs=wt_sb[:, fc * O:(fc + 1) * O],
                                start=(fc == 0), stop=(fc == 3))
                        if j % 2 == 0:
                            nc.vector.tensor_scalar_mul(
                                g_sb[:, j * O:(j + 1) * O], pfw[:],
                                hdt_sb[:, j:j + 1])
                        else:
                            nc.scalar.mul(
                                g_sb[:, j * O:(j + 1) * O], pfw[:],
                                hdt_sb[:, j:j + 1])

                    if jb == 0:
                        # e = (hat_d_own * g_own)^T; own tiles are j =
                        # 0..MT-1, all inside block 0. Runs while later
                        # blocks stream in.
                        for jj in range(MT):
                            for h in range(2):
                                sc = scp.tile([128, 128], F32, tag="sc")
                                nc.vector.tensor_scalar_mul(
                                    sc[:],
                                    g_sb[:, jj * O + h * 128:
                                         jj * O + (h + 1) * 128],
                                    hdt_sb[:, jj:jj + 1])
                                ptp = ps1.tile([128, 128], F32, tag="tp",
                                               bufs=2)
                                nc.tensor.transpose(ptp[:], sc[:], ident[:])
                                nc.vector.tensor_copy(
                                    e_sb[:, h * SH + jj * 128:
                                         h * SH + (jj + 1) * 128],
                                    ptp[:])

            # epilogue-only constants: queued between the feature stream and
            # the A stream; done long before the epilogue needs them.
            b_sb = constp.tile([128, 2], F32, tag="b")
            for h in range(2):
                nc.scalar.dma_start(out=b_sb[:, h:h + 1],
                                    in_=bvec[h * 128:(h + 1) * 128, :])
            # hat_d of own rows broadcast across all partitions (free dim = m)
            hd_bc = constp.tile([128, SH], F32, tag="hdbc")
            nc.scalar.dma_start(out=hd_bc[:],
                                in_=hdo[0:1, :].to_broadcast((128, SH)))

            # ---- main: acc[h] = (A_sh @ g)^T for o-half h ----
            with tc.tile_pool(name="ps2", bufs=1, space="PSUM") as psp:
                accs = [psp.tile([128, SH], F32, tag=f"acc{h}", name=f"acc{h}")
                        for h in range(2)]
                for k in range(JT):
                    sl = asp.tile([128, SH], F16, tag="as")
                    qs[k % 2].dma_start(out=sl[:],
                                        in_=at[k * 128:(k + 1) * 128, :])
                    for h in range(2):
                        lhs = g_sb[:, k * O + h * 128:k * O + (h + 1) * 128]
                        for mc in range(4):
                            nc.tensor.matmul(
                                accs[h][:, mc * 512:(mc + 1) * 512],
                                lhsT=lhs,
                                rhs=sl[:, mc * 512:(mc + 1) * 512],
                                start=(k == 0), stop=(k == JT - 1))

                # ---- epilogue: out^T = hat_d_own * acc + b + e ----
                for h in range(2):
                    for c in range(4):
                        cs = slice(c * 512, (c + 1) * 512)
                        t = wp.tile([128, 512], F32, tag="t")
                        nc.vector.tensor_tensor(t[:], accs[h][:, cs],
                                                hd_bc[:, cs], mult)
                        nc.vector.scalar_tensor_tensor(
                            t[:], in0=t[:], scalar=b_sb[:, h:h + 1],
                            in1=e_sb[:, h * SH + c * 512:
                                     h * SH + (c + 1) * 512],
                            op0=add, op1=add)
                        qs[(h + c) % 2].dma_start(
                            out=outT[h * 128:(h + 1) * 128, cs], in_=t[:])

    nc.compile()
    return nc


def prep_inputs(A, hat_d, feature, W, b):
    """Per-core input maps. Host work is layout/dtype prep only: transpose,
    slice, concatenate (the own-rows-first node permutation on the j axis),
    and the fp32->fp16 dtype conversion for matmul operands."""
    A = np.ascontiguousarray(np.asarray(A, dtype=np.float32))
    hat_d = np.ascontiguousarray(np.asarray(hat_d, dtype=np.float32))
    feature = np.ascontiguousarray(np.asarray(feature, dtype=np.float32))
    W = np.asarray(W, dtype=np.float32)
    b = np.asarray(b, dtype=np.float32)

    featT = np.ascontiguousarray(feature.T.astype(np.float16))  # [F, N]
    wt = np.ascontiguousarray(W.T.astype(np.float16))  # [F, O]
    b2 = np.ascontiguousarray(b.reshape(O, 1))

    in_maps = []
    for c in range(NCORES):
        r0, r1 = c * SH, (c + 1) * SH
        rows = A[r0:r1].astype(np.float16)  # [SH, N]
        # A_sh^T with node (j) axis permuted own-rows-first
        at_c = np.empty((N, SH), dtype=np.float16)
        at_c[:SH] = rows[:, r0:r1].T
        at_c[SH:SH + r0] = rows[:, :r0].T
        at_c[SH + r0:] = rows[:, r1:].T

        ft_c = np.empty((F, N), dtype=np.float16)
        ft_c[:, :SH] = featT[:, r0:r1]
        ft_c[:, SH:SH + r0] = featT[:, :r0]
        ft_c[:, SH + r0:] = featT[:, r1:]

        hd_c = np.concatenate([hat_d[r0:r1], hat_d[:r0], hat_d[r1:]])
        hdt_c = np.ascontiguousarray(hd_c.reshape(JT, 128).T)
        hdo_c = np.ascontiguousarray(hat_d[r0:r1].reshape(1, SH))

        in_maps.append({
            "at": at_c,
            "ft": ft_c,
            "hdt": hdt_c,
            "hdo": hdo_c,
            "wt": wt,
            "bvec": b2,
        })
    return in_maps


last_exec_time_ns = None
last_results = None


def kernel(A, hat_d, feature, W, b):
    global last_exec_time_ns, last_results
    if "nc" not in _CACHE:
        _CACHE["nc"] = build_program()
    nc = _CACHE["nc"]

    in_maps = prep_inputs(A, hat_d, feature, W, b)
    trace = bool(int(os.environ.get("KERNEL_TRACE", "0")))
    res = run_bass_kernel_spmd(nc, in_maps, list(range(NCORES)), trace=trace)
    last_exec_time_ns = res.exec_time_ns
    last_results = res

    out = np.empty((N, O), dtype=np.float32)
    for c in range(NCORES):
        out[c * SH:(c + 1) * SH] = res.results[c]["outT"].T
    return out
